# revision 30
# baseline (speedup 1.0000x reference)
"""MoE routed-expert kernel for Trainium2 (8 NeuronCores, SPMD).

Problem: N=16384 tokens, D=768, H=768, C=2, E=20 experts.
  y[n] = relu(x[n] @ W1[e] + b1[e]) @ W2[e] + b2[e],  e = component_idx[n]

Host: sort tokens by expert, split the 20 groups into 24 fragments, deal
into 8 cores x 3 "expert slots" with uniform per-slot capacity (SPMD:
every core runs the same static program; the host stages each slot's
expert weights/tokens). Slot order = [middle, largest, smallest]: the
smallest slot plus a 128-token tail chunk minimize the exit chain.

Device, per core (v2 schedule):
 - All payload DMAs ride the two HWDGE rings (scalar + sync sequencers),
   balanced by bytes and emitted in consumption order. Slot0 chunk0 is
   dt-major; its six (w slab, x piece) pairs alternate rings with the
   dt=0 pair split in half across both rings so round 0 unblocks at the
   DMA cold-start latency floor. Later chunks use one strided x DMA
   each; slot1/2 weights stream as two 3-dt pieces on opposite rings,
   all issued well ahead of PE need.
 - PE warmup: fp16 matmuls on uninitialized SBUF (values are never
   read) ramp the HAM clock while the first slabs are in flight.
 - Layer 1: 6x6 accumulating fp16 matmuls per chunk (<=512 tokens).
 - relu+bias: PSUM->SBUF fp16, split across Vector and Scalar engines.
 - Layer 2 (C=2): 6 accumulating [128->2] matmuls into a [2,T] PSUM
   bank (2 column groups), bias added on Vector; emitted two L1 rounds
   into the next chunk so the PE never waits on the last relu.
"""

import math

import numpy as np

import concourse.bass as bass  # noqa: F401
import concourse.mybir as mybir
from concourse import bacc
from concourse.bass_utils import run_bass_kernel_spmd
from concourse.tile import TileContext

F32 = mybir.dt.float32
F16 = mybir.dt.float16
MM_DT = F16
MM_NP = np.float16

N_CORES = 8
N_SLOTS = 3
D = 768
H = 768
C = 2
DT = D // 128  # 6 d-tiles
HT = H // 128  # 6 h-tiles
MAX_CHUNK = 512  # one PSUM bank holds 512 fp32 -> matmul free dim cap
TAIL = 128  # final chunk size (shortens the exit chain)

N_WARMUP = 31  # fp16 [128,128] dummy matmuls until the first slabs land

ADD = mybir.AluOpType.add
MAX_OP = mybir.AluOpType.max
RELU = mybir.ActivationFunctionType.Relu


def _chunk_sizes(
    cap: int, tail_split: bool = False, front_load: bool = False
) -> list[int]:
    if front_load and MAX_CHUNK < cap <= 2 * MAX_CHUNK and cap - MAX_CHUNK >= 2:
        # big first chunk: the dt-major fill phase needs one 196KB w slab
        # per round regardless of chunk size, so longer rounds lower the
        # required early HBM rate
        sizes = [MAX_CHUNK, cap - MAX_CHUNK]
    else:
        n = max(1, math.ceil(cap / MAX_CHUNK))
        base = (cap // n) & ~1
        sizes = [base] * n
        sizes[-1] = cap - base * (n - 1)
    if tail_split and sizes[-1] > 2 * TAIL:
        sizes[-1:] = [sizes[-1] - TAIL, TAIL]
    assert sum(sizes) == cap and all(s % 2 == 0 and 0 < s <= MAX_CHUNK for s in sizes)
    return sizes


def _plan_packing(counts: np.ndarray):
    """Return (caps, assign): per-slot capacities and
    assign[s][c] = (expert, start_within_group, length).
    Slot order: [middle..., largest, smallest]."""
    frags = [(int(e), 0, int(c)) for e, c in enumerate(counts) if c > 0]
    target = N_CORES * N_SLOTS
    assert len(frags) <= target, (
        f"{len(frags)} non-empty experts exceed {target} slots; raise N_SLOTS"
    )
    while len(frags) < target:
        frags.sort(key=lambda f: -f[2])
        e, st, ln = frags[0]
        if ln < 2:
            frags.append((e, st, 0))
            continue
        h1 = ln // 2
        frags[0] = (e, st, ln - h1)
        frags.append((e, st + (ln - h1), h1))
    frags.sort(key=lambda f: -f[2])
    groups = []
    for s in range(N_SLOTS):
        group = frags[s * N_CORES : (s + 1) * N_CORES]
        cap = max(2, max(f[2] for f in group))
        cap += cap % 2
        groups.append((cap, group))
    # middle slots first, then largest, smallest last
    groups.sort(key=lambda g: -g[0])
    largest, smallest = groups[0], groups[-1]
    middle = groups[1:-1]
    groups = middle + [largest, smallest]
    return [g[0] for g in groups], [g[1] for g in groups]


_PROGRAM_CACHE: dict = {}


def _build_program(caps: tuple):
    if caps in _PROGRAM_CACHE:
        return _PROGRAM_CACHE[caps]

    R = sum(caps)
    offs = [0]
    for c in caps[:-1]:
        offs.append(offs[-1] + c)
    chunk_plan = []  # (s, co, size, last_of_slot)
    for s in range(N_SLOTS):
        sizes = _chunk_sizes(
            caps[s], tail_split=(s == N_SLOTS - 1), front_load=(s == 0)
        )
        co = 0
        for ci, size in enumerate(sizes):
            chunk_plan.append((s, co, size, ci == len(sizes) - 1))
            co += size
    n_chunks = len(chunk_plan)

    nc = bacc.Bacc(
        "TRN2", target_bir_lowering=False, debug=False, num_devices=N_CORES
    )
    xTp = nc.dram_tensor("xTp", [128, DT, R], MM_DT, kind="ExternalInput")
    w1 = nc.dram_tensor("w1", [N_SLOTS, 128, DT, H], MM_DT, kind="ExternalInput")
    b1 = nc.dram_tensor("b1", [128, N_SLOTS, HT], F32, kind="ExternalInput")
    w2 = nc.dram_tensor("w2", [128, N_SLOTS, HT, C], MM_DT, kind="ExternalInput")
    b2 = nc.dram_tensor("b2", [C, N_SLOTS], F32, kind="ExternalInput")
    y = nc.dram_tensor("y", [C, R], F32, kind="ExternalOutput")

    s0_sizes = _chunk_sizes(caps[0], front_load=True)
    c0 = s0_sizes[0]

    with TileContext(nc) as tc:
        with (
            tc.tile_pool(name="inpool", bufs=1) as inpool,
            tc.tile_pool(name="hpool", bufs=3) as hpool,
            tc.tile_pool(name="pspool", bufs=7, space="PSUM") as pspool,
            tc.tile_pool(name="pypool", bufs=1, space="PSUM") as pypool,
        ):
            # Warmup operand (content irrelevant): memset on gpsimd. The
            # profiler's exec window opens at the framework's own const
            # memsets (~6.4us, gpsimd) regardless, so placement of this
            # one is not timing-sensitive.
            wu_w = inpool.tile([128, 128], MM_DT, name="wu_w", tag="wu_w")
            nc.gpsimd.memset(wu_w[:, :], 0.0)

            # ---- DMA schedule ----
            # Each engine rotates its dma_starts over 4 HW queues with ONE
            # outstanding transfer per queue; queue k's first transfer
            # starts ~(1.4us cold start + 0.7us * k issue serialization)
            # after the sequencer enters main. So the first four issues
            # per engine define the fill-phase landings: x0d0 halves ride
            # queue 1 of each engine (land first), w0d0 halves queue 2;
            # later rounds pair (w, x) across engines. Everything needed
            # after ~18us rides sync only — the scalar sequencer must be
            # free for relu work from ~15us on.
            s0w = [
                inpool.tile([128, H], MM_DT, name=f"w0d{dt}", tag=f"w0d{dt}")
                for dt in range(DT)
            ]
            cap0 = caps[0]
            s0x = [
                inpool.tile([128, cap0], MM_DT, name=f"x0d{dt}", tag=f"x0d{dt}")
                for dt in range(DT)
            ]
            h0 = (c0 // 2) & ~1
            nc.sync.dma_start(out=s0x[0][:, h0:c0], in_=xTp[:, 0, h0:c0])
            nc.scalar.dma_start(out=s0x[0][:, 0:h0], in_=xTp[:, 0, 0:h0])
            nc.sync.dma_start(out=s0w[0][:, H // 2 : H], in_=w1[0, :, 0, H // 2 : H])
            nc.scalar.dma_start(out=s0w[0][:, 0 : H // 2], in_=w1[0, :, 0, 0 : H // 2])
            # dt=1..5: w and x chunk-0 pieces on opposite rings,
            # alternating; round 1's w slab grabs the next queue slot
            # ahead of b1 (b1 is tiny and only needed by the first relu)
            for dt in range(1, DT):
                r_w = nc.sync if dt % 2 == 1 else nc.scalar
                r_x = nc.scalar if dt % 2 == 1 else nc.sync
                r_w.dma_start(out=s0w[dt], in_=w1[0, :, dt, :])
                r_x.dma_start(out=s0x[dt][:, 0:c0], in_=xTp[:, dt, 0:c0])
                if dt == 1:
                    b1_sb = inpool.tile(
                        [128, N_SLOTS, HT], F32, name="b1_sb", tag="b1"
                    )
                    nc.sync.dma_start(out=b1_sb, in_=b1[:, :, :])
            # slot0 chunk-1 per-dt x pieces: all on scalar (it drains by
            # ~16us anyway), freeing sync issue slots so slot1/2 pieces
            # start ~2us earlier
            for ci in range(1, len(s0_sizes)):
                lo = sum(s0_sizes[:ci])
                hi = lo + s0_sizes[ci]
                for dt in range(DT):
                    nc.scalar.dma_start(
                        out=s0x[dt][:, lo:hi], in_=xTp[:, dt, lo:hi]
                    )
            # w2/b2 (needed by the first L2, ~chunk1): last scalar issues
            w2_sb = inpool.tile([128, N_SLOTS, HT, C], MM_DT, name="w2_sb", tag="w2")
            nc.scalar.dma_start(out=w2_sb, in_=w2[:, :, :, :])
            b2_sb = inpool.tile([C, N_SLOTS], F32, name="b2_sb", tag="b2")
            nc.scalar.dma_start(out=b2_sb, in_=b2[:, :])

            # slots 1..: everything on the sync ring, in strict need
            # order. A slot's w pieces (3 x 2dt) interleave with its
            # first x chunk's pieces (2 x 3dt) — both are needed at the
            # slot's first matmul — so neither starves the other on the
            # fair-shared queues. Later x chunks follow whole. Keep the
            # piece count low: extra concurrent DMA write streams slow
            # the PE's SBUF reads measurably.
            sx = {}
            sw = {}
            for s in range(1, N_SLOTS):
                sw[s] = inpool.tile(
                    [128, DT, H], MM_DT, name=f"wslot{s}", tag=f"wslot{s}"
                )
                szs = [z for (ss, _, z, _) in chunk_plan if ss == s]
                for co_, sz_ in zip([0] + list(np.cumsum(szs[:-1])), szs):
                    sx[(s, int(co_))] = inpool.tile(
                        [128, DT, sz_], MM_DT, name=f"x{s}_{co_}", tag=f"x{s}_{co_}"
                    )

            def x_piece(s, co, sz, d0, d1, ring=None):
                (ring or nc.sync).dma_start(
                    out=sx[(s, co)][:, d0:d1, :],
                    in_=xTp[:, d0:d1, offs[s] + co : offs[s] + co + sz],
                )

            # slot1 on sync: first chunk's w+x pieces interleaved
            s1szs = [z for (ss, _, z, _) in chunk_plan if ss == 1]
            s1cos = [0] + list(int(c) for c in np.cumsum(s1szs[:-1]))
            nc.sync.dma_start(out=sw[1][:, 0:2, :], in_=w1[1, :, 0:2, :])
            x_piece(1, s1cos[0], s1szs[0], 0, 3)
            nc.sync.dma_start(out=sw[1][:, 2:4, :], in_=w1[1, :, 2:4, :])
            x_piece(1, s1cos[0], s1szs[0], 3, DT)
            nc.sync.dma_start(out=sw[1][:, 4:DT, :], in_=w1[1, :, 4:DT, :])
            for co_, sz_ in zip(s1cos[1:], s1szs[1:]):
                x_piece(1, co_, sz_, 0, DT)
            # slot2 on sync too: concurrent DMA streams on extra queues
            # push the chip's power governor to a lower core clock level
            # (~-17% on EVERY engine), so mid-stream stays single-ring.
            s2szs = [z for (ss, _, z, _) in chunk_plan if ss == 2]
            s2cos = [0] + list(int(c) for c in np.cumsum(s2szs[:-1]))
            nc.sync.dma_start(out=sw[2][:, 0:2, :], in_=w1[2, :, 0:2, :])
            x_piece(2, s2cos[0], s2szs[0], 0, 3)
            nc.sync.dma_start(out=sw[2][:, 2:4, :], in_=w1[2, :, 2:4, :])
            x_piece(2, s2cos[0], s2szs[0], 3, DT)
            nc.sync.dma_start(out=sw[2][:, 4:DT, :], in_=w1[2, :, 4:DT, :])
            for co_, sz_ in zip(s2cos[1:], s2szs[1:]):
                x_piece(2, co_, sz_, 0, DT)

            # PE warmup: flips the HAM clock gate before real data lands
            wu_ps = pypool.tile([128, 128], F32, name="wu_ps", tag="psy")
            for _ in range(N_WARMUP):
                nc.tensor.matmul(wu_ps, wu_w, wu_w, start=True, stop=True)

            y_slot = [
                inpool.tile([C, caps[s]], F32, name=f"ysb{s}", tag=f"ysb{s}")
                for s in range(N_SLOTS)
            ]

            # ---- main loop ----
            # Chunk i's layer-2 + fold are emitted AFTER the first two
            # ht-rounds of chunk i+1's layer-1, so the PE never waits on
            # chunk i's last relu (it drains during those rounds).
            def emit_l2(s, co, size, last, is_final):
                ps_y = pypool.tile([128, size], F32, name="ps_y", tag="psy")
                n_grp = 1 if is_final else 2
                h_sb = h_of[(s, co)]
                for ht in range(HT):
                    g = ht % n_grp
                    nc.tensor.matmul(
                        ps_y[32 * g : 32 * g + C, :],
                        w2_sb[:, s, ht, :],
                        h_sb[:, ht, :],
                        start=(ht < n_grp),
                        stop=(ht >= HT - n_grp),
                        tile_position=(0, 32 * g),
                    )
                nc.vector.tensor_scalar_add(
                    y_slot[s][:, co : co + size],
                    ps_y[0:C, :],
                    b2_sb[:, s : s + 1],
                )
                if n_grp == 2:
                    nc.vector.tensor_tensor(
                        y_slot[s][:, co : co + size],
                        y_slot[s][:, co : co + size],
                        ps_y[32 : 32 + C, :],
                        op=ADD,
                    )
                if s == N_SLOTS - 1:
                    # sync ring: its DMA path is warm at the end (a cold
                    # scalar-ring DMA here measured ~1us slower)
                    nc.sync.dma_start(
                        out=y[:, offs[s] + co : offs[s] + co + size],
                        in_=y_slot[s][:, co : co + size],
                    )
                elif last:
                    nc.sync.dma_start(
                        out=y[:, offs[s] : offs[s] + caps[s]],
                        in_=y_slot[s][:, 0 : caps[s]],
                    )

            h_of = {}
            pending = None  # (s, co, size, last)
            for idx, (s, co, size, last) in enumerate(chunk_plan):
                is_final = idx == n_chunks - 1
                h_sb = hpool.tile([128, HT, size], MM_DT, name="h_sb", tag="h")
                h_of[(s, co)] = h_sb
                ps_list = [
                    pspool.tile([128, size], F32, name=f"ps_h{ht}", tag="psh")
                    for ht in range(HT)
                ]
                if s == 0 and co == 0:
                    # dt-major: round dt needs only that dt's slab+piece
                    rounds = [
                        [
                            (ps_list[ht], s0w[dt][:, ht * 128 : (ht + 1) * 128],
                             s0x[dt][:, 0:size], dt == 0, dt == DT - 1)
                            for ht in range(HT)
                        ]
                        for dt in range(DT)
                    ]
                elif s == 0:
                    rounds = [
                        [
                            (ps_list[ht], s0w[dt][:, ht * 128 : (ht + 1) * 128],
                             s0x[dt][:, co : co + size], dt == 0, dt == DT - 1)
                            for dt in range(DT)
                        ]
                        for ht in range(HT)
                    ]
                else:
                    xt = sx[(s, co)]
                    rounds = [
                        [
                            (ps_list[ht], sw[s][:, dt, ht * 128 : (ht + 1) * 128],
                             xt[:, dt, :], dt == 0, dt == DT - 1)
                            for dt in range(DT)
                        ]
                        for ht in range(HT)
                    ]
                for ri, rnd in enumerate(rounds):
                    if ri == 2 and pending is not None:
                        emit_l2(*pending[:3], pending[3], False)
                        pending = None
                    for ps, lhsT, rhs, st, sp in rnd:
                        nc.tensor.matmul(ps, lhsT, rhs, start=st, stop=sp)
                # relu+bias, split across both elementwise engines
                for ht in range(HT):
                    if ht % 2 == 0:
                        nc.vector.tensor_scalar(
                            h_sb[:, ht, :],
                            ps_list[ht],
                            b1_sb[:, s, ht : ht + 1],
                            0.0,
                            op0=ADD,
                            op1=MAX_OP,
                        )
                    else:
                        nc.scalar.activation(
                            h_sb[:, ht, :],
                            ps_list[ht],
                            RELU,
                            bias=b1_sb[:, s, ht : ht + 1],
                        )
                pending = (s, co, size, last)
            emit_l2(*pending[:3], pending[3], True)

    nc.compile()
    _PROGRAM_CACHE[caps] = nc
    return nc


def kernel(embeddings, component_idx, W1, b1, W2, b2):
    embeddings = np.ascontiguousarray(np.asarray(embeddings, dtype=np.float32))
    ci = np.asarray(component_idx).astype(np.int64, copy=False)
    W1 = np.asarray(W1, dtype=np.float32)
    b1 = np.asarray(b1, dtype=np.float32)
    W2 = np.asarray(W2, dtype=np.float32)
    b2 = np.asarray(b2, dtype=np.float32)

    N = embeddings.shape[0]
    E = W1.shape[0]

    counts = np.bincount(ci, minlength=E)
    order = np.argsort(ci, kind="stable")
    group_start = np.zeros(E, dtype=np.int64)
    group_start[1:] = np.cumsum(counts)[:-1]
    x_sorted = embeddings[order]  # [N, D] grouped by expert

    caps, assign = _plan_packing(counts)
    R = sum(caps)
    offs = np.cumsum([0] + caps[:-1]).tolist() if len(caps) > 1 else [0]

    nc = _build_program(tuple(caps))

    # host-side packing
    w1_packed = np.ascontiguousarray(
        W1.reshape(E, DT, 128, H).transpose(0, 2, 1, 3)
    ).astype(MM_NP)  # [e, p, dt, h]
    b1_packed = np.ascontiguousarray(
        b1.reshape(E, HT, 128).transpose(0, 2, 1)
    )  # [e, 128, ht]
    w2_packed = np.ascontiguousarray(
        W2.reshape(E, HT, 128, C).transpose(0, 2, 1, 3)
    ).astype(MM_NP)  # [e, p, ht, c]

    in_maps = []
    for c in range(N_CORES):
        Xc = np.zeros((R, D), dtype=MM_NP)
        w1_in = np.empty((N_SLOTS, 128, DT, H), dtype=MM_NP)
        b1_in = np.empty((128, N_SLOTS, HT), dtype=np.float32)
        w2_in = np.empty((128, N_SLOTS, HT, C), dtype=MM_NP)
        b2_in = np.empty((C, N_SLOTS), dtype=np.float32)
        for s in range(N_SLOTS):
            e, st, ln = assign[s][c]
            beg = group_start[e] + st
            Xc[offs[s] : offs[s] + ln] = x_sorted[beg : beg + ln]
            w1_in[s] = w1_packed[e]
            b1_in[:, s, :] = b1_packed[e]
            w2_in[:, s, :, :] = w2_packed[e]
            b2_in[:, s] = b2[e]
        xTp_in = np.ascontiguousarray(Xc.T.reshape(DT, 128, R).transpose(1, 0, 2))
        im = {"xTp": xTp_in, "w1": w1_in, "b1": b1_in, "w2": w2_in, "b2": b2_in}
        in_maps.append(im)

    global _LAST_IN_MAPS
    _LAST_IN_MAPS = in_maps
    res = run_bass_kernel_spmd(nc, in_maps, list(range(N_CORES)))

    out = np.empty((N, C), dtype=np.float32)
    for c in range(N_CORES):
        yc = res.results[c]["y"]  # [C, R]
        for s in range(N_SLOTS):
            e, st, ln = assign[s][c]
            beg = group_start[e] + st
            tokens = order[beg : beg + ln]
            out[tokens] = yc[:, offs[s] : offs[s] + ln].T
    return out


# revision 31
# speedup vs baseline: 1.1859x; 1.1859x over previous
"""MoE routed-expert kernel for Trainium2 (8 NeuronCores, SPMD).

Problem: N=16384 tokens, D=768, H=768, C=2, E=20 experts.
  y[n] = relu(x[n] @ W1[e] + b1[e]) @ W2[e] + b2[e],  e = component_idx[n]

Host: sort tokens by expert, split the 20 groups into 24 fragments, deal
into 8 cores x 3 "expert slots" with uniform per-slot capacity (SPMD:
every core runs the same static program; the host stages each slot's
expert weights/tokens). Slot order = [middle, largest, smallest]: the
smallest slot plus a 128-token tail chunk minimize the exit chain.

Device, per core (v2 schedule):
 - All payload DMAs ride the two HWDGE rings (scalar + sync sequencers),
   balanced by bytes and emitted in consumption order. Slot0 chunk0 is
   dt-major; its six (w slab, x piece) pairs alternate rings with the
   dt=0 pair split in half across both rings so round 0 unblocks at the
   DMA cold-start latency floor. Later chunks use one strided x DMA
   each; slot1/2 weights stream as two 3-dt pieces on opposite rings,
   all issued well ahead of PE need.
 - PE warmup: fp16 matmuls on uninitialized SBUF (values are never
   read) ramp the HAM clock while the first slabs are in flight.
 - Layer 1: 6x6 accumulating fp16 matmuls per chunk (<=512 tokens).
 - relu+bias: PSUM->SBUF fp16, split across Vector and Scalar engines.
 - Layer 2 (C=2): 6 accumulating [128->2] matmuls into a [2,T] PSUM
   bank (2 column groups), bias added on Vector; emitted two L1 rounds
   into the next chunk so the PE never waits on the last relu.
"""

import math

import numpy as np

import concourse.bass as bass  # noqa: F401
import concourse.mybir as mybir
from concourse import bacc
from concourse.bass_utils import run_bass_kernel_spmd
from concourse.tile import TileContext

F32 = mybir.dt.float32
F16 = mybir.dt.float16
MM_DT = F16
MM_NP = np.float16

N_CORES = 8
N_SLOTS = 3
D = 768
H = 768
C = 2
DT = D // 128  # 6 d-tiles
HT = H // 128  # 6 h-tiles
MAX_CHUNK = 512  # one PSUM bank holds 512 fp32 -> matmul free dim cap
TAIL = 128  # final chunk size (shortens the exit chain)

N_WARMUP = 31  # fp16 [128,128] dummy matmuls until the first slabs land

ADD = mybir.AluOpType.add
MAX_OP = mybir.AluOpType.max
RELU = mybir.ActivationFunctionType.Relu


def _chunk_sizes(
    cap: int, tail_split: bool = False, front_load: bool = False
) -> list[int]:
    if front_load and MAX_CHUNK < cap <= 2 * MAX_CHUNK and cap - MAX_CHUNK >= 2:
        # big first chunk: the dt-major fill phase needs one 196KB w slab
        # per round regardless of chunk size, so longer rounds lower the
        # required early HBM rate
        sizes = [MAX_CHUNK, cap - MAX_CHUNK]
    else:
        n = max(1, math.ceil(cap / MAX_CHUNK))
        base = (cap // n) & ~1
        sizes = [base] * n
        sizes[-1] = cap - base * (n - 1)
    if tail_split and sizes[-1] > 2 * TAIL:
        sizes[-1:] = [sizes[-1] - TAIL, TAIL]
    assert sum(sizes) == cap and all(s % 2 == 0 and 0 < s <= MAX_CHUNK for s in sizes)
    return sizes


def _plan_packing(counts: np.ndarray):
    """Return (caps, assign): per-slot capacities and
    assign[s][c] = (expert, start_within_group, length).
    Slot order: [middle..., largest, smallest]."""
    frags = [(int(e), 0, int(c)) for e, c in enumerate(counts) if c > 0]
    target = N_CORES * N_SLOTS
    assert len(frags) <= target, (
        f"{len(frags)} non-empty experts exceed {target} slots; raise N_SLOTS"
    )
    while len(frags) < target:
        frags.sort(key=lambda f: -f[2])
        e, st, ln = frags[0]
        if ln < 2:
            frags.append((e, st, 0))
            continue
        h1 = ln // 2
        frags[0] = (e, st, ln - h1)
        frags.append((e, st + (ln - h1), h1))
    frags.sort(key=lambda f: -f[2])
    groups = []
    for s in range(N_SLOTS):
        group = frags[s * N_CORES : (s + 1) * N_CORES]
        cap = max(2, max(f[2] for f in group))
        cap += cap % 2
        groups.append((cap, group))
    # middle slots first, then largest, smallest last
    groups.sort(key=lambda g: -g[0])
    largest, smallest = groups[0], groups[-1]
    middle = groups[1:-1]
    groups = middle + [largest, smallest]
    return [g[0] for g in groups], [g[1] for g in groups]


_PROGRAM_CACHE: dict = {}


def _build_program(caps: tuple):
    if caps in _PROGRAM_CACHE:
        return _PROGRAM_CACHE[caps]

    R = sum(caps)
    offs = [0]
    for c in caps[:-1]:
        offs.append(offs[-1] + c)
    chunk_plan = []  # (s, co, size, last_of_slot)
    for s in range(N_SLOTS):
        sizes = _chunk_sizes(
            caps[s], tail_split=(s == N_SLOTS - 1), front_load=(s == 0)
        )
        co = 0
        for ci, size in enumerate(sizes):
            chunk_plan.append((s, co, size, ci == len(sizes) - 1))
            co += size
    n_chunks = len(chunk_plan)

    nc = bacc.Bacc(
        "TRN2", target_bir_lowering=False, debug=False, num_devices=N_CORES
    )
    xTp = nc.dram_tensor("xTp", [128, DT, R], MM_DT, kind="ExternalInput")
    w1 = nc.dram_tensor("w1", [N_SLOTS, 128, DT, H], MM_DT, kind="ExternalInput")
    b1 = nc.dram_tensor("b1", [128, N_SLOTS, HT], F32, kind="ExternalInput")
    w2 = nc.dram_tensor("w2", [128, N_SLOTS, HT, C], MM_DT, kind="ExternalInput")
    b2 = nc.dram_tensor("b2", [C, N_SLOTS], F32, kind="ExternalInput")
    y = nc.dram_tensor("y", [C, R], F32, kind="ExternalOutput")

    s0_sizes = _chunk_sizes(caps[0], front_load=True)
    c0 = s0_sizes[0]

    with TileContext(nc) as tc:
        with (
            tc.tile_pool(name="inpool", bufs=1) as inpool,
            tc.tile_pool(name="hpool", bufs=3) as hpool,
            tc.tile_pool(name="pspool", bufs=7, space="PSUM") as pspool,
            tc.tile_pool(name="pypool", bufs=1, space="PSUM") as pypool,
        ):
            # Warmup operand (content irrelevant): memset on gpsimd. The
            # profiler's exec window opens at the framework's own const
            # memsets (~6.4us, gpsimd) regardless, so placement of this
            # one is not timing-sensitive.
            wu_w = inpool.tile([128, 128], MM_DT, name="wu_w", tag="wu_w")
            nc.gpsimd.memset(wu_w[:, :], 0.0)

            # ---- DMA schedule ----
            # Each engine rotates its dma_starts over 4 HW queues with ONE
            # outstanding transfer per queue; queue k's first transfer
            # starts ~(1.4us cold start + 0.7us * k issue serialization)
            # after the sequencer enters main. So the first four issues
            # per engine define the fill-phase landings: x0d0 halves ride
            # queue 1 of each engine (land first), w0d0 halves queue 2;
            # later rounds pair (w, x) across engines. Everything needed
            # after ~18us rides sync only — the scalar sequencer must be
            # free for relu work from ~15us on.
            s0w = [
                inpool.tile([128, H], MM_DT, name=f"w0d{dt}", tag=f"w0d{dt}")
                for dt in range(DT)
            ]
            cap0 = caps[0]
            s0x = [
                inpool.tile([128, cap0], MM_DT, name=f"x0d{dt}", tag=f"x0d{dt}")
                for dt in range(DT)
            ]
            h0 = (c0 // 2) & ~1
            nc.sync.dma_start(out=s0x[0][:, h0:c0], in_=xTp[:, 0, h0:c0])
            nc.scalar.dma_start(out=s0x[0][:, 0:h0], in_=xTp[:, 0, 0:h0])
            nc.sync.dma_start(out=s0w[0][:, H // 2 : H], in_=w1[0, :, 0, H // 2 : H])
            nc.scalar.dma_start(out=s0w[0][:, 0 : H // 2], in_=w1[0, :, 0, 0 : H // 2])
            # dt=1..5: w and x chunk-0 pieces on opposite rings,
            # alternating; round 1's slabs grab the next queue slots
            # ahead of b1 (tiny, only needed by the first relu ~17us)
            for dt in range(1, DT):
                r_w = nc.sync if dt % 2 == 1 else nc.scalar
                r_x = nc.scalar if dt % 2 == 1 else nc.sync
                r_w.dma_start(out=s0w[dt], in_=w1[0, :, dt, :])
                r_x.dma_start(out=s0x[dt][:, 0:c0], in_=xTp[:, dt, 0:c0])
                if dt == 1:
                    b1_sb = inpool.tile(
                        [128, N_SLOTS, HT], F32, name="b1_sb", tag="b1"
                    )
                    nc.sync.dma_start(out=b1_sb, in_=b1[:, :, :])
            # slot0 chunk-1 per-dt x pieces, alternating rings
            for ci in range(1, len(s0_sizes)):
                lo = sum(s0_sizes[:ci])
                hi = lo + s0_sizes[ci]
                for dt in range(DT):
                    r_x = nc.sync if dt % 2 == 0 else nc.scalar
                    r_x.dma_start(out=s0x[dt][:, lo:hi], in_=xTp[:, dt, lo:hi])
            # w2/b2 (needed by the first L2, ~chunk1): last scalar issues
            w2_sb = inpool.tile([128, N_SLOTS, HT, C], MM_DT, name="w2_sb", tag="w2")
            nc.scalar.dma_start(out=w2_sb, in_=w2[:, :, :, :])
            b2_sb = inpool.tile([C, N_SLOTS], F32, name="b2_sb", tag="b2")
            nc.scalar.dma_start(out=b2_sb, in_=b2[:, :])

            # slots 1..: everything on the sync ring, in strict need
            # order. A slot's w pieces (3 x 2dt) interleave with its
            # first x chunk's pieces (2 x 3dt) — both are needed at the
            # slot's first matmul — so neither starves the other on the
            # fair-shared queues. Later x chunks follow whole. Keep the
            # piece count low: extra concurrent DMA write streams slow
            # the PE's SBUF reads measurably.
            sx = {}
            sw = {}
            for s in range(1, N_SLOTS):
                sw[s] = inpool.tile(
                    [128, DT, H], MM_DT, name=f"wslot{s}", tag=f"wslot{s}"
                )
                szs = [z for (ss, _, z, _) in chunk_plan if ss == s]
                for co_, sz_ in zip([0] + list(np.cumsum(szs[:-1])), szs):
                    sx[(s, int(co_))] = inpool.tile(
                        [128, DT, sz_], MM_DT, name=f"x{s}_{co_}", tag=f"x{s}_{co_}"
                    )

            def x_piece(s, co, sz, d0, d1, ring=None):
                (ring or nc.sync).dma_start(
                    out=sx[(s, co)][:, d0:d1, :],
                    in_=xTp[:, d0:d1, offs[s] + co : offs[s] + co + sz],
                )

            # slot1 on sync: first chunk's w+x pieces interleaved
            s1szs = [z for (ss, _, z, _) in chunk_plan if ss == 1]
            s1cos = [0] + list(int(c) for c in np.cumsum(s1szs[:-1]))
            nc.sync.dma_start(out=sw[1][:, 0:2, :], in_=w1[1, :, 0:2, :])
            x_piece(1, s1cos[0], s1szs[0], 0, 3)
            nc.sync.dma_start(out=sw[1][:, 2:4, :], in_=w1[1, :, 2:4, :])
            x_piece(1, s1cos[0], s1szs[0], 3, DT)
            nc.sync.dma_start(out=sw[1][:, 4:DT, :], in_=w1[1, :, 4:DT, :])
            for co_, sz_ in zip(s1cos[1:], s1szs[1:]):
                x_piece(1, co_, sz_, 0, DT)
            # slot2 on sync too: concurrent DMA streams on extra queues
            # push the chip's power governor to a lower core clock level
            # (~-17% on EVERY engine), so mid-stream stays single-ring.
            s2szs = [z for (ss, _, z, _) in chunk_plan if ss == 2]
            s2cos = [0] + list(int(c) for c in np.cumsum(s2szs[:-1]))
            nc.sync.dma_start(out=sw[2][:, 0:2, :], in_=w1[2, :, 0:2, :])
            x_piece(2, s2cos[0], s2szs[0], 0, 3)
            nc.sync.dma_start(out=sw[2][:, 2:4, :], in_=w1[2, :, 2:4, :])
            x_piece(2, s2cos[0], s2szs[0], 3, DT)
            nc.sync.dma_start(out=sw[2][:, 4:DT, :], in_=w1[2, :, 4:DT, :])
            for co_, sz_ in zip(s2cos[1:], s2szs[1:]):
                x_piece(2, co_, sz_, 0, DT)

            # PE warmup: flips the HAM clock gate before real data lands
            wu_ps = pypool.tile([128, 128], F32, name="wu_ps", tag="psy")
            for _ in range(N_WARMUP):
                nc.tensor.matmul(wu_ps, wu_w, wu_w, start=True, stop=True)

            y_slot = [
                inpool.tile([C, caps[s]], F32, name=f"ysb{s}", tag=f"ysb{s}")
                for s in range(N_SLOTS)
            ]

            # ---- main loop ----
            # Chunk i's layer-2 + fold are emitted AFTER the first two
            # ht-rounds of chunk i+1's layer-1, so the PE never waits on
            # chunk i's last relu (it drains during those rounds).
            def emit_l2(s, co, size, last, is_final):
                ps_y = pypool.tile([128, size], F32, name="ps_y", tag="psy")
                n_grp = 1 if is_final else 2
                h_sb = h_of[(s, co)]
                for ht in range(HT):
                    g = ht % n_grp
                    nc.tensor.matmul(
                        ps_y[32 * g : 32 * g + C, :],
                        w2_sb[:, s, ht, :],
                        h_sb[:, ht, :],
                        start=(ht < n_grp),
                        stop=(ht >= HT - n_grp),
                        tile_position=(0, 32 * g),
                    )
                nc.vector.tensor_scalar_add(
                    y_slot[s][:, co : co + size],
                    ps_y[0:C, :],
                    b2_sb[:, s : s + 1],
                )
                if n_grp == 2:
                    nc.vector.tensor_tensor(
                        y_slot[s][:, co : co + size],
                        y_slot[s][:, co : co + size],
                        ps_y[32 : 32 + C, :],
                        op=ADD,
                    )
                if s == N_SLOTS - 1:
                    # sync ring: its DMA path is warm at the end (a cold
                    # scalar-ring DMA here measured ~1us slower)
                    nc.sync.dma_start(
                        out=y[:, offs[s] + co : offs[s] + co + size],
                        in_=y_slot[s][:, co : co + size],
                    )
                elif last:
                    nc.sync.dma_start(
                        out=y[:, offs[s] : offs[s] + caps[s]],
                        in_=y_slot[s][:, 0 : caps[s]],
                    )

            h_of = {}
            pending = None  # (s, co, size, last)
            for idx, (s, co, size, last) in enumerate(chunk_plan):
                is_final = idx == n_chunks - 1
                h_sb = hpool.tile([128, HT, size], MM_DT, name="h_sb", tag="h")
                h_of[(s, co)] = h_sb
                ps_list = [
                    pspool.tile([128, size], F32, name=f"ps_h{ht}", tag="psh")
                    for ht in range(HT)
                ]
                if s == 0 and co == 0:
                    # dt-major: round dt needs only that dt's slab+piece
                    rounds = [
                        [
                            (ps_list[ht], s0w[dt][:, ht * 128 : (ht + 1) * 128],
                             s0x[dt][:, 0:size], dt == 0, dt == DT - 1)
                            for ht in range(HT)
                        ]
                        for dt in range(DT)
                    ]
                elif s == 0:
                    rounds = [
                        [
                            (ps_list[ht], s0w[dt][:, ht * 128 : (ht + 1) * 128],
                             s0x[dt][:, co : co + size], dt == 0, dt == DT - 1)
                            for dt in range(DT)
                        ]
                        for ht in range(HT)
                    ]
                else:
                    xt = sx[(s, co)]
                    rounds = [
                        [
                            (ps_list[ht], sw[s][:, dt, ht * 128 : (ht + 1) * 128],
                             xt[:, dt, :], dt == 0, dt == DT - 1)
                            for dt in range(DT)
                        ]
                        for ht in range(HT)
                    ]
                for ri, rnd in enumerate(rounds):
                    if ri == 2 and pending is not None:
                        emit_l2(*pending[:3], pending[3], False)
                        pending = None
                    for ps, lhsT, rhs, st, sp in rnd:
                        nc.tensor.matmul(ps, lhsT, rhs, start=st, stop=sp)
                # relu+bias, split across both elementwise engines
                for ht in range(HT):
                    if ht % 2 == 0:
                        nc.vector.tensor_scalar(
                            h_sb[:, ht, :],
                            ps_list[ht],
                            b1_sb[:, s, ht : ht + 1],
                            0.0,
                            op0=ADD,
                            op1=MAX_OP,
                        )
                    else:
                        nc.scalar.activation(
                            h_sb[:, ht, :],
                            ps_list[ht],
                            RELU,
                            bias=b1_sb[:, s, ht : ht + 1],
                        )
                pending = (s, co, size, last)
            emit_l2(*pending[:3], pending[3], True)

    nc.compile()
    _PROGRAM_CACHE[caps] = nc
    return nc


def kernel(embeddings, component_idx, W1, b1, W2, b2):
    embeddings = np.ascontiguousarray(np.asarray(embeddings, dtype=np.float32))
    ci = np.asarray(component_idx).astype(np.int64, copy=False)
    W1 = np.asarray(W1, dtype=np.float32)
    b1 = np.asarray(b1, dtype=np.float32)
    W2 = np.asarray(W2, dtype=np.float32)
    b2 = np.asarray(b2, dtype=np.float32)

    N = embeddings.shape[0]
    E = W1.shape[0]

    counts = np.bincount(ci, minlength=E)
    order = np.argsort(ci, kind="stable")
    group_start = np.zeros(E, dtype=np.int64)
    group_start[1:] = np.cumsum(counts)[:-1]
    x_sorted = embeddings[order]  # [N, D] grouped by expert

    caps, assign = _plan_packing(counts)
    R = sum(caps)
    offs = np.cumsum([0] + caps[:-1]).tolist() if len(caps) > 1 else [0]

    nc = _build_program(tuple(caps))

    # host-side packing
    w1_packed = np.ascontiguousarray(
        W1.reshape(E, DT, 128, H).transpose(0, 2, 1, 3)
    ).astype(MM_NP)  # [e, p, dt, h]
    b1_packed = np.ascontiguousarray(
        b1.reshape(E, HT, 128).transpose(0, 2, 1)
    )  # [e, 128, ht]
    w2_packed = np.ascontiguousarray(
        W2.reshape(E, HT, 128, C).transpose(0, 2, 1, 3)
    ).astype(MM_NP)  # [e, p, ht, c]

    in_maps = []
    for c in range(N_CORES):
        Xc = np.zeros((R, D), dtype=MM_NP)
        w1_in = np.empty((N_SLOTS, 128, DT, H), dtype=MM_NP)
        b1_in = np.empty((128, N_SLOTS, HT), dtype=np.float32)
        w2_in = np.empty((128, N_SLOTS, HT, C), dtype=MM_NP)
        b2_in = np.empty((C, N_SLOTS), dtype=np.float32)
        for s in range(N_SLOTS):
            e, st, ln = assign[s][c]
            beg = group_start[e] + st
            Xc[offs[s] : offs[s] + ln] = x_sorted[beg : beg + ln]
            w1_in[s] = w1_packed[e]
            b1_in[:, s, :] = b1_packed[e]
            w2_in[:, s, :, :] = w2_packed[e]
            b2_in[:, s] = b2[e]
        xTp_in = np.ascontiguousarray(Xc.T.reshape(DT, 128, R).transpose(1, 0, 2))
        im = {"xTp": xTp_in, "w1": w1_in, "b1": b1_in, "w2": w2_in, "b2": b2_in}
        in_maps.append(im)

    global _LAST_IN_MAPS
    _LAST_IN_MAPS = in_maps
    res = run_bass_kernel_spmd(nc, in_maps, list(range(N_CORES)))

    out = np.empty((N, C), dtype=np.float32)
    for c in range(N_CORES):
        yc = res.results[c]["y"]  # [C, R]
        for s in range(N_SLOTS):
            e, st, ln = assign[s][c]
            beg = group_start[e] + st
            tokens = order[beg : beg + ln]
            out[tokens] = yc[:, offs[s] : offs[s] + ln].T
    return out
